# revision 1
# baseline (speedup 1.0000x reference)
"""DLightGCN (LightGCN propagation + disentangled-factor scoring) on 8 trn2
NeuronCores via Bass/Tile.

Sharding: edge list and segment-sum sharded by destination-node partition
(core i owns padded node rows [i*R, (i+1)*R)); the per-layer node features are
exchanged with an on-device AllGather; factor weights are replicated and the
(user,item) batch is data-parallel across cores.

Per destination tile of 128 rows, edges are packed into chunks of 128 (one
edge per SBUF partition). Each chunk: an indirect DMA gathers the 128 source
rows (one per partition); a fused DVE tensor_scalar builds the selection
matrix valhot[e, r] = (iota[r]==localrow[e]) * val[e] from a constant iota
tile; the tensor engine accumulates valhot.T @ gathered into PSUM across the
tile's chunks, which yields the segment-sum for those 128 destination rows.

kernel(**inputs) takes the FULL problem inputs and returns the FULL [B]
scores; all sharding happens inside.
"""
import sys
from dataclasses import dataclass

import numpy as np

for _p in ("/opt/trn_rl_repo", "/root/.axon_site/_ro/trn_rl_repo"):
    if _p not in sys.path:
        sys.path.append(_p)

import concourse.bass as bass  # noqa: E402
import concourse.mybir as mybir  # noqa: E402
from concourse.bass import IndirectOffsetOnAxis  # noqa: E402

F32 = mybir.dt.float32
I32 = mybir.dt.int32
AF = mybir.ActivationFunctionType
OP = mybir.AluOpType


@dataclass
class Cfg:
    n_cores: int = 8
    D: int = 128
    K: int = 4
    L: int = 3
    T: int = 147          # dest tiles per core
    CPT: int = 35         # 128-edge chunks per dest tile
    BT: int = 16          # batch tiles (of 128 pairs) per core

    @property
    def R(self):
        return self.T * 128

    @property
    def N_PAD(self):
        return self.n_cores * self.R

    @property
    def TC(self):
        return self.T * self.CPT


def body(tc, outs, ins, cfg: Cfg):
    nc = tc.nc
    D, K, T, CPT, BT = cfg.D, cfg.K, cfg.T, cfg.CPT, cfg.BT
    x0 = ins["x0"]
    acc0 = ins["acc0"]
    scores = outs["scores"]
    rg = [list(range(cfg.n_cores))]

    with tc.tile_pool(name="dram", bufs=1, space="DRAM") as dpool:
        acc_buf = dpool.tile([cfg.R, D], F32)
        y1 = dpool.tile([cfg.R, D], F32)
        y2 = dpool.tile([cfg.R, D], F32)
        light_sl = dpool.tile([cfg.R, D], F32)
        x1f = dpool.tile([cfg.N_PAD, D], F32, addr_space="Shared")
        x2f = dpool.tile([cfg.N_PAD, D], F32, addr_space="Shared")
        lightf = dpool.tile([cfg.N_PAD, D], F32, addr_space="Shared")

        with (
            tc.tile_pool(name="cpool", bufs=1) as cpool,
            tc.tile_pool(name="gpool", bufs=24) as gpool,
            tc.tile_pool(name="vpool", bufs=24) as vpool,
            tc.tile_pool(name="pspool", bufs=4, space="PSUM") as pspool,
            tc.tile_pool(name="epool", bufs=6) as epool,
        ):
            idx_sb = cpool.tile([128, cfg.TC], I32)
            nc.sync.dma_start(idx_sb[:], ins["src_idx"][:])
            lr_sb = cpool.tile([128, cfg.TC], F32)
            nc.sync.dma_start(lr_sb[:], ins["lr"][:])
            ev_sb = cpool.tile([128, cfg.TC], F32)
            nc.sync.dma_start(ev_sb[:], ins["ev"][:])
            iota_sb = cpool.tile([128, 128], F32)
            nc.sync.dma_start(iota_sb[:], ins["iota"][:])

            xs = [x0, x1f, x2f]
            accs = [acc0, acc_buf, acc_buf]
            ys = [y1, y2, None]
            for layer in range(cfg.L):
                xsrc = xs[layer]
                for t in range(T):
                    ps = pspool.tile([128, D], F32, name=f"ps_{layer}_{t}", tag="ps")
                    for c in range(CPT):
                        cc = t * CPT + c
                        g = gpool.tile([128, D], F32, name=f"g_{layer}_{t}_{c}", tag="g")
                        nc.gpsimd.indirect_dma_start(
                            out=g[:], out_offset=None, in_=xsrc[:],
                            in_offset=IndirectOffsetOnAxis(
                                ap=idx_sb[:, cc:cc + 1], axis=0))
                        vh = vpool.tile([128, 128], F32, name=f"vh_{layer}_{t}_{c}", tag="vh")
                        # scalar_tensor_tensor (2-tensor form) stays in DVE 1x
                        # mode: no 2-port perf-mode lock on the shared
                        # DVE<->GpSimd SBUF port, which the Q7 SWDGE needs for
                        # its descriptor rings while generating the gathers.
                        nc.vector.scalar_tensor_tensor(
                            out=vh[:], in0=iota_sb[:],
                            scalar=lr_sb[:, cc:cc + 1],
                            in1=ev_sb[:, cc:cc + 1].to_broadcast([128, 128]),
                            op0=OP.is_equal, op1=OP.mult)
                        nc.tensor.matmul(
                            ps[:], lhsT=vh[:], rhs=g[:],
                            start=(c == 0), stop=(c == CPT - 1))
                    acc_old = epool.tile([128, D], F32, name=f"ao_{layer}_{t}", tag="ao")
                    nc.sync.dma_start(acc_old[:], accs[layer][t * 128:(t + 1) * 128, :])
                    if layer < cfg.L - 1:
                        ynew = epool.tile([128, D], F32, name=f"yn_{layer}_{t}", tag="yn")
                        nc.vector.tensor_copy(ynew[:], ps[:])
                        nc.sync.dma_start(ys[layer][t * 128:(t + 1) * 128, :], ynew[:])
                        accn = epool.tile([128, D], F32, name=f"an_{layer}_{t}", tag="an")
                        nc.vector.tensor_tensor(out=accn[:], in0=acc_old[:], in1=ps[:], op=OP.add)
                        nc.sync.dma_start(acc_buf[t * 128:(t + 1) * 128, :], accn[:])
                    else:
                        lt = epool.tile([128, D], F32, name=f"lt_{t}", tag="yn")
                        nc.vector.tensor_scalar(
                            out=lt[:], in0=acc_old[:], scalar1=1.0 / (cfg.L + 1),
                            scalar2=None, op0=OP.mult)
                        nc.vector.scalar_tensor_tensor(
                            out=lt[:], in0=ps[:], scalar=1.0 / (cfg.L + 1),
                            in1=lt[:], op0=OP.mult, op1=OP.add)
                        nc.sync.dma_start(light_sl[t * 128:(t + 1) * 128, :], lt[:])
                if layer == 0:
                    nc.gpsimd.collective_compute(
                        "AllGather", OP.bypass, ins=[y1.opt()], outs=[x1f.opt()],
                        replica_groups=rg)
                elif layer == 1:
                    nc.gpsimd.collective_compute(
                        "AllGather", OP.bypass, ins=[y2.opt()], outs=[x2f.opt()],
                        replica_groups=rg)
            nc.gpsimd.collective_compute(
                "AllGather", OP.bypass, ins=[light_sl.opt()], outs=[lightf.opt()],
                replica_groups=rg)

        # ---- batch stage ----
        with (
            tc.tile_pool(name="bcpool", bufs=1) as bcpool,
            tc.tile_pool(name="bpool", bufs=3) as bpool,
            tc.tile_pool(name="bps", bufs=2, space="PSUM") as bps,
        ):
            wft_sb = bcpool.tile([128, K * D], F32)
            nc.sync.dma_start(wft_sb[:], ins["wft"][:])
            bias_sb = bcpool.tile([128, K * D], F32)
            nc.sync.dma_start(bias_sb[:], ins["bias"][:])
            ws_sb = bcpool.tile([128, K * K], F32)
            nc.sync.dma_start(ws_sb[:], ins["ws"][:])
            ident_sb = bcpool.tile([128, 128], F32)
            nc.sync.dma_start(ident_sb[:], ins["identity"][:])
            u_idx = bcpool.tile([128, BT], I32)
            nc.sync.dma_start(u_idx[:], ins["users_idx"][:])
            i_idx = bcpool.tile([128, BT], I32)
            nc.sync.dma_start(i_idx[:], ins["items_idx"][:])
            sc = bcpool.tile([128, BT], F32)

            for tb in range(BT):
                fs = []
                for side, sidx in (("u", u_idx), ("i", i_idx)):
                    e = bpool.tile([128, D], F32, name=f"e{side}_{tb}", tag=f"e{side}")
                    nc.gpsimd.indirect_dma_start(
                        out=e[:], out_offset=None, in_=lightf[:],
                        in_offset=IndirectOffsetOnAxis(ap=sidx[:, tb:tb + 1], axis=0))
                    pt = bps.tile([128, 128], F32, name=f"pt{side}_{tb}", tag="pt")
                    nc.tensor.transpose(pt[:], e[:], ident_sb[:])
                    eT = bpool.tile([128, D], F32, name=f"eT{side}_{tb}", tag=f"eT{side}")
                    nc.vector.tensor_copy(eT[:], pt[:])
                    fp = bps.tile([128, K * D], F32, name=f"fp{side}_{tb}", tag="fp")
                    nc.tensor.matmul(fp[:], lhsT=eT[:], rhs=wft_sb[:], start=True, stop=True)
                    f = bpool.tile([128, K * D], F32, name=f"f{side}_{tb}", tag=f"f{side}")
                    nc.vector.tensor_tensor(out=f[:], in0=fp[:], in1=bias_sb[:], op=OP.add)
                    nc.vector.tensor_scalar(out=f[:], in0=f[:], scalar1=0.0,
                                            scalar2=None, op0=OP.max)
                    n2 = bpool.tile([128, K], F32, name=f"n2{side}_{tb}", tag=f"n2{side}")
                    sq = bpool.tile([128, K * D], F32, name=f"sq{side}_{tb}", tag="sq")
                    nc.vector.tensor_tensor(out=sq[:], in0=f[:], in1=f[:], op=OP.mult)
                    for k in range(K):
                        nc.vector.reduce_sum(out=n2[:, k:k + 1],
                                             in_=sq[:, k * D:(k + 1) * D],
                                             axis=mybir.AxisListType.X)
                    nc.vector.tensor_scalar(out=n2[:], in0=n2[:], scalar1=1e-24,
                                            scalar2=None, op0=OP.max)
                    nrm = bpool.tile([128, K], F32, name=f"nr{side}_{tb}", tag=f"nr{side}")
                    nc.scalar.activation(nrm[:], n2[:], AF.Sqrt)
                    inv = bpool.tile([128, K], F32, name=f"iv{side}_{tb}", tag=f"iv{side}")
                    nc.vector.reciprocal(inv[:], nrm[:])
                    for k in range(K):
                        nc.vector.tensor_scalar(
                            out=f[:, k * D:(k + 1) * D], in0=f[:, k * D:(k + 1) * D],
                            scalar1=inv[:, k:k + 1], scalar2=None, op0=OP.mult)
                    fs.append(f)
                uf, itf = fs
                h = bpool.tile([128, K * K], F32, name=f"h_{tb}", tag="h")
                pr = bpool.tile([128, D], F32, name=f"pr_{tb}", tag="pr")
                for i in range(K):
                    for j in range(K):
                        nc.vector.tensor_tensor(
                            out=pr[:], in0=uf[:, i * D:(i + 1) * D],
                            in1=itf[:, j * D:(j + 1) * D], op=OP.mult)
                        nc.vector.reduce_sum(out=h[:, i * K + j:i * K + j + 1],
                                             in_=pr[:], axis=mybir.AxisListType.X)
                hw = bpool.tile([128, K * K], F32, name=f"hw_{tb}", tag="hw")
                nc.vector.tensor_tensor(out=hw[:], in0=h[:], in1=ws_sb[:], op=OP.mult)
                nc.vector.reduce_sum(out=sc[:, tb:tb + 1], in_=hw[:],
                                     axis=mybir.AxisListType.X)
            nc.sync.dma_start(scores[:], sc[:])


def build_full(cfg: Cfg):
    import concourse.bacc as bacc
    import concourse.tile as tile_mod
    nc = bacc.Bacc("TRN2", target_bir_lowering=False, debug=False,
                   num_devices=cfg.n_cores)
    shapes = dict(
        x0=([cfg.N_PAD, 128], F32), acc0=([cfg.R, 128], F32),
        src_idx=([128, cfg.TC], I32),
        lr=([128, cfg.TC], F32), ev=([128, cfg.TC], F32),
        iota=([128, 128], F32), identity=([128, 128], F32),
        wft=([128, 512], F32), bias=([128, 512], F32), ws=([128, 16], F32),
        users_idx=([128, cfg.BT], I32), items_idx=([128, cfg.BT], I32),
    )
    ins = {k: nc.dram_tensor(k, s, d, kind="ExternalInput").ap()
           for k, (s, d) in shapes.items()}
    outs = {"scores": nc.dram_tensor("scores", [128, cfg.BT], F32,
                                     kind="ExternalOutput").ap()}
    with tile_mod.TileContext(nc) as tc:
        body(tc, outs, ins, cfg)
    nc.compile()
    return nc


def host_prepare(inputs, cfg: Cfg):
    users = np.asarray(inputs["users"])
    items = np.asarray(inputs["items"])
    edge_index = np.asarray(inputs["edge_index"])
    edge_vals = np.asarray(inputs["edge_vals"], dtype=np.float32)
    user_emb = np.asarray(inputs["user_emb"], dtype=np.float32)
    item_emb = np.asarray(inputs["item_emb"], dtype=np.float32)
    W_f = np.asarray(inputs["W_f"], dtype=np.float32)
    b_f = np.asarray(inputs["b_f"], dtype=np.float32)
    W_s = np.asarray(inputs["W_s"], dtype=np.float32)

    n_users = user_emb.shape[0]
    N = n_users + item_emb.shape[0]
    D, K, T, CPT, BT, NC = cfg.D, cfg.K, cfg.T, cfg.CPT, cfg.BT, cfg.n_cores
    assert N <= cfg.N_PAD, (N, cfg.N_PAD)
    B = users.shape[0]
    assert B == NC * BT * 128, (B, NC, BT)

    all_emb = np.zeros((cfg.N_PAD, D), np.float32)
    all_emb[:n_users] = user_emb
    all_emb[n_users:N] = item_emb

    rows = edge_index[0].astype(np.int64)
    cols = edge_index[1].astype(np.int64)
    order = np.argsort(rows, kind="stable")
    rs, cs, vs = rows[order], cols[order], edge_vals[order]
    gt = rs // 128
    counts = np.bincount(gt, minlength=NC * T)
    need = int(np.ceil(counts.max() / 128))
    assert need <= CPT, f"CPT too small: need {need}, have {CPT}"
    starts = np.zeros(NC * T, np.int64)
    starts[1:] = np.cumsum(counts)[:-1]
    pos = np.arange(len(rs)) - starts[gt]
    chunk = pos // 128
    lane = pos % 128
    core_of = gt // T
    t_in_core = gt % T
    colidx = t_in_core * CPT + chunk

    src = np.zeros((NC, 128, cfg.TC), np.int32)
    lr = np.zeros((NC, 128, cfg.TC), np.float32)
    ev = np.zeros((NC, 128, cfg.TC), np.float32)
    src[core_of, lane, colidx] = cs
    lr[core_of, lane, colidx] = (rs % 128).astype(np.float32)
    ev[core_of, lane, colidx] = vs

    iota = np.tile(np.arange(128, dtype=np.float32), (128, 1))
    ident = np.eye(128, dtype=np.float32)
    wft = np.transpose(W_f, (2, 0, 1)).reshape(D, K * D).copy()
    bias = np.tile(b_f.reshape(1, K * D), (128, 1)).astype(np.float32)
    ws = np.tile(W_s.reshape(1, K * K), (128, 1)).astype(np.float32)

    in_maps = []
    for c in range(NC):
        u_sh = users[c * BT * 128:(c + 1) * BT * 128].astype(np.int32)
        i_sh = items[c * BT * 128:(c + 1) * BT * 128].astype(np.int32) + n_users
        in_maps.append(dict(
            x0=all_emb,
            acc0=all_emb[c * cfg.R:(c + 1) * cfg.R].copy(),
            src_idx=src[c], lr=lr[c], ev=ev[c],
            iota=iota, identity=ident, wft=wft, bias=bias, ws=ws,
            users_idx=np.ascontiguousarray(u_sh.reshape(BT, 128).T),
            items_idx=np.ascontiguousarray(i_sh.reshape(BT, 128).T),
        ))
    return in_maps


def host_post(results, cfg: Cfg):
    outs = []
    for c in range(cfg.n_cores):
        arr = results[c]["scores"]  # [128, BT]
        outs.append(arr.T.reshape(-1))
    return np.concatenate(outs)


_CACHE = {}


def kernel(**inputs) -> np.ndarray:
    from concourse import bass_utils

    # CPT must cover the fullest destination tile; prescan the edge rows.
    base = Cfg()
    rows = np.asarray(inputs["edge_index"])[0].astype(np.int64)
    counts = np.bincount(rows // 128, minlength=base.n_cores * base.T)
    need = int(np.ceil(counts.max() / 128))
    cfg = Cfg(CPT=max(need, 1))
    in_maps = host_prepare(inputs, cfg)
    _CACHE["cfg"] = cfg
    nc = _CACHE.get(("nc", cfg.CPT))
    if nc is None:
        nc = build_full(cfg)
        _CACHE[("nc", cfg.CPT)] = nc
    res = bass_utils.run_bass_kernel_spmd(
        nc, in_maps, core_ids=list(range(cfg.n_cores)))
    return host_post(res.results, cfg).astype(np.float32)

